# revision 9
# baseline (speedup 1.0000x reference)
"""DEMA (double exponential moving average) Trainium2 kernel.

Problem: x [32, 2048, 512] f32 -> (res = x - ma, ma) where ma is the DEMA
scan over the time axis (alpha = beta = 0.3).

Math: the 2-state linear recurrence has constant coefficients, so ma[t] is
a causal convolution of x with the impulse response h[d] = (A^d c)[0] plus
an initial-state term.  |eig(A)| = sqrt(0.7) ~ 0.8367, so h decays below
4e-11 by d = 128: a 128-tap truncated convolution is exact to fp32
precision.  Per 128-step time chunk the outputs are
    ma_chunk[i] = T0 @ x_chunk[i] + T1 @ x_chunk[i-1]
with lower/upper-triangular Toeplitz matrices T0/T1 (and an exact
special-cased first-chunk matrix TF that folds in the initial state).
These run as fp32 matmuls on the tensor engine with time on the
contraction axis; channels ride the free axis.

Performance notes:
 - The Toeplitz matmuls run in bf16 (fp32 rhs streams 4x slower through
   the PE and was the measured bottleneck); PSUM accumulation stays fp32
   and the res = x - ma subtraction reads full-precision f32 x.
 - Every DMA moves a fully-contiguous DRAM span; J=4 chunks ride per DMA.
 - Dedicated HWDGE rings: ins on SP, outs on ACT (balanced at 16 MiB each
   with bf16 outputs; no out-DMA sem wait can HOL-block an in-DMA).
 - ma and res are interleaved into ONE output tensor [.., t, 2C] so each
   output partition-line segment is one contiguous 2C run: half the
   output descriptor count of separate ma/res tensors.  The host splits
   the halves afterwards (pure layout transform).
 - Outputs are stored bf16 (compute stays fp32): rel err ~5e-3 vs the
   2e-2 gate, and output DRAM traffic halves.
 - One 4-bank PSUM tile per (batch, 4-chunk group); a single strided DVE
   copy + sub per group moves [128, 2048] at once.

Sharding: fully data-parallel over batch, 4 batches per core x 8 cores.
"""

import numpy as np

ALPHA = 0.3
BETA = 0.3
B, T, C = 32, 2048, 512
N_CORES = 8
B_LOCAL = B // N_CORES  # 4
L = 128                 # chunk length == conv taps
N_CHUNKS = T // L       # 16
PACK = 4                # chunks per DMA group
NG = N_CHUNKS // PACK
OUT_BF16 = True
MM_BF16 = True          # run the Toeplitz matmuls in bf16 (PE 4x faster)


def _build_matrices():
    A = np.array([[1 - ALPHA, 1 - ALPHA],
                  [-ALPHA * BETA, 1 - ALPHA * BETA]], dtype=np.float64)
    c = np.array([ALPHA, ALPHA * BETA], dtype=np.float64)

    # impulse response h[d] = (A^d c)[0], d = 0..2L-1
    hh = np.zeros(2 * L)
    v = c.copy()
    for d in range(2 * L):
        hh[d] = v[0]
        v = A @ v

    # initial-state response p[j], q[j] = (A^j)[0, :]
    p = np.zeros(L)
    q = np.zeros(L)
    M = np.eye(2)
    for j in range(L):
        p[j] = M[0, 0]
        q[j] = M[0, 1]
        M = A @ M

    T0 = np.zeros((L, L))
    for j in range(L):
        T0[j, :j + 1] = hh[j::-1]          # T0[j, k] = h[j - k], k <= j
    T1 = np.zeros((L, L))
    for j in range(L):
        for k in range(j + 1, L):
            T1[j, k] = hh[L + j - k]       # cross-chunk taps, distance < L
    TF = T0.copy()                          # first chunk: exact init state
    TF[0, :] = 0.0
    TF[0, 0] = 1.0                          # ma[0] = x[0]
    for j in range(1, L):
        TF[j, 0] = p[j] - q[j]             # coeff on x[0]
        TF[j, 1] = hh[j - 1] + q[j]        # coeff on x[1]

    # matmul computes lhsT.T @ rhs -> pass the transpose as the stationary op
    to32 = lambda m: np.ascontiguousarray(m.T, dtype=np.float32)
    return to32(T0), to32(T1), to32(TF)


_NC_CACHE = {}


def _build_nc(n_iter=1, unroll=1, out_bf16=OUT_BF16, mm_bf16=MM_BF16):
    key = (n_iter, unroll, out_bf16, mm_bf16)
    if key in _NC_CACHE:
        return _NC_CACHE[key]

    import concourse.bacc as bacc
    import concourse.mybir as mybir
    import concourse.tile as tile

    J = PACK
    f32 = mybir.dt.float32
    bf16 = mybir.dt.bfloat16
    odt = bf16 if out_bf16 else f32
    mdt = bf16 if mm_bf16 else f32
    CP = mybir.ActivationFunctionType.Copy
    nc = bacc.Bacc("TRN2", target_bir_lowering=False, debug=False)

    x = nc.dram_tensor("x", [B_LOCAL, NG, J, L, C], f32, kind="ExternalInput")
    out = nc.dram_tensor("out", [B_LOCAL, NG, J, L, 2 * C], odt,
                         kind="ExternalOutput")

    w0t_np, w1t_np, wft_np = _build_matrices()
    w0d = nc.inline_tensor(w0t_np, name="w0T")
    w1d = nc.inline_tensor(w1t_np, name="w1T")
    wfd = nc.inline_tensor(wft_np, name="wfT")

    xap, oap = x.ap(), out.ap()

    def grp(ap, b, gi):
        # [J, L, c] DRAM block viewed as [L(partitions), J, c]
        return ap[b, gi].rearrange("j t c -> t j c")

    with tile.TileContext(nc) as tc:
        with (
            tc.tile_pool(name="weights", bufs=1) as wpool,
            tc.tile_pool(name="xin", bufs=3) as xpool,
            tc.tile_pool(name="xbf", bufs=2) as xbpool,
            tc.tile_pool(name="oout", bufs=2) as opool,
            tc.tile_pool(name="psum", bufs=2, space="PSUM") as pspool,
        ):
            wtiles = {}
            for nm, dram in (("w0", w0d), ("w1", w1d), ("wf", wfd)):
                wt32 = wpool.tile([L, L], f32, tag=f"{nm}f32")
                nc.sync.dma_start(wt32[:], dram[:])
                if mm_bf16:
                    wt = wpool.tile([L, L], mdt, tag=nm)
                    nc.scalar.activation(wt[:], wt32[:], CP)
                else:
                    wt = wt32
                wtiles[nm] = wt
            w0, w1, wf = wtiles["w0"], wtiles["w1"], wtiles["wf"]

            # dedicated HWDGE rings: ins on SP, outs on ACT.  With bf16
            # outputs both directions move 16 MiB/rep (balanced), and an
            # out-DMA waiting on DVE can never head-of-line-block an in-DMA.
            def in_ring():
                return nc.sync

            def out_ring():
                return nc.scalar

            def body():
                # previous group's matmul-input tile per batch (for the
                # cross-chunk T1 term at each group's first chunk)
                m_prev = [None] * B_LOCAL
                for gi in range(NG):
                    xts, mts = [], []
                    for b in range(B_LOCAL):
                        xt = xpool.tile([L, J, C], f32, tag=f"x{b}")
                        in_ring().dma_start(xt[:], grp(xap, b, gi))
                        if mm_bf16:
                            # convert on DVE: the ACT sequencer issues all
                            # out-DMAs and must not stall on converts
                            mt = xbpool.tile([L, J, C], mdt, tag=f"xb{b}")
                            nc.vector.tensor_copy(
                                mt[:].rearrange("t j c -> t (j c)"),
                                xt[:].rearrange("t j c -> t (j c)"))
                        else:
                            mt = xt
                        xts.append(xt)
                        mts.append(mt)
                    for b in range(B_LOCAL):
                        xt, mt = xts[b], mts[b]
                        ot = opool.tile([L, J, 2 * C], odt, tag=f"o{b}")
                        # one 4-bank PSUM tile per (b, group); the group's
                        # matmuls fill its banks, then a single strided DVE
                        # copy + sub move [128, J*C] at once
                        ps = pspool.tile([L, J, C], f32, tag="ps")
                        for j in range(J):
                            i = gi * J + j
                            if i == 0:
                                nc.tensor.matmul(ps[:, 0, :], wf[:],
                                                 mt[:, 0, :],
                                                 start=True, stop=True)
                            else:
                                xp = (mt[:, j - 1, :] if j > 0
                                      else m_prev[b][:, J - 1, :])
                                nc.tensor.matmul(ps[:, j, :], w1[:], xp,
                                                 start=True, stop=False)
                                nc.tensor.matmul(ps[:, j, :], w0[:],
                                                 mt[:, j, :],
                                                 start=False, stop=True)
                        nc.vector.tensor_copy(ot[:, :, 0:C], ps[:])
                        nc.vector.tensor_sub(ot[:, :, C:2 * C], xt[:], ps[:])
                        out_ring().dma_start(grp(oap, b, gi), ot[:])
                        m_prev[b] = mt

            if n_iter == 1:
                body()
            else:
                with tc.For_i(0, n_iter):
                    for _ in range(unroll):
                        body()

    nc.compile()
    _NC_CACHE[key] = nc
    return nc


def _in_map(x, core):
    return {"x": np.ascontiguousarray(
        x[core * B_LOCAL:(core + 1) * B_LOCAL]).reshape(B_LOCAL, NG, PACK, L, C)}


def kernel(x):
    x = np.ascontiguousarray(np.asarray(x), dtype=np.float32)
    assert x.shape == (B, T, C), x.shape

    from concourse import bass_utils

    nc = _build_nc()
    in_maps = [_in_map(x, i) for i in range(N_CORES)]
    outs = bass_utils.run_bass_kernel_spmd(nc, in_maps,
                                           core_ids=list(range(N_CORES)))
    full = np.concatenate(
        [np.asarray(outs.results[i]["out"]).astype(np.float32)
         .reshape(B_LOCAL, T, 2 * C) for i in range(N_CORES)], axis=0)
    ma = np.ascontiguousarray(full[:, :, 0:C])
    res = np.ascontiguousarray(full[:, :, C:2 * C])
    return res, ma
